# revision 13
# baseline (speedup 1.0000x reference)
"""Multi-head attention + output projection on 8 Trainium2 NeuronCores.

Problem: B=8, S=1024, E=1024, H=16 heads (d=64), fp32.
  out[b] = softmax(Q K^T / sqrt(d) + mask_bias) V @ W^T

Sharding: data parallel — batch b runs on core b. Each core computes a
full single-batch attention + projection.

Per-core algorithm (all fp32):
  - Load Q^T, K^T (head-dim-major, [E,S] layout) and W^T via transposed
    DRAM access patterns; V natural.
  - For each head pair (2p, 2p+1), for each q-half (qc) and k-block (kb):
      S_T[k,q] = K_h Q_h^T via row-packed matmul pairs (contraction=64
      per head, two heads packed in the 128-row PE array).
      P = exp(S_T * 0.125 + mask_bias)  (one [128,1024] ACT op; the mask
      bias is per-partition because partitions are k here).
      O_T[c,q] += V_kb^T-free matmul, two heads col-packed.
      D[q]   += ones^T P (softmax denominator), col-packed M=1 matmuls.
    Normalize: O_T *= broadcast(1/D) (K=2 matmul broadcast + DVE mul).
  - Projection: Y[q,f] = sum_eb OTs_eb^T Wt_eb, accumulated in PSUM.
"""

import os
import numpy as np

import concourse.bass as bass
import concourse.mybir as mybir
import concourse.tile as tile
from concourse import bacc
from concourse.bass_utils import run_bass_kernel_spmd

B, S, E, H = 8, 1024, 1024, 16
D = E // H  # 64 head dim
NCORES = 8
FP32 = mybir.dt.float32
KB = S // 128  # 8 k-blocks
QCHUNK = 512


def emit(tc: tile.TileContext):
    nc = tc.nc
    # qT/kT are [E, S] (head-dim major), wT is [E, F] — transposed on host.
    q_d = nc.dram_tensor("qT", [E, S], FP32, kind="ExternalInput").ap()
    k_d = nc.dram_tensor("kT", [E, S], FP32, kind="ExternalInput").ap()
    v_d = nc.dram_tensor("v", [S, E], FP32, kind="ExternalInput").ap()
    m_d = nc.dram_tensor("m", [S], mybir.dt.uint8, kind="ExternalInput").ap()
    w_d = nc.dram_tensor("wT", [E, E], FP32, kind="ExternalInput").ap()
    o_d = nc.dram_tensor("out", [S, E], FP32, kind="ExternalOutput").ap()

    with (
        tc.tile_pool(name="consts", bufs=1) as consts,
        tc.tile_pool(name="bigq", bufs=1) as bigq,
        tc.tile_pool(name="p", bufs=4) as p_pool,
        tc.tile_pool(name="ots", bufs=10) as ots_pool,
        tc.tile_pool(name="ysb", bufs=2) as ysb_pool,
        tc.tile_pool(name="rps", bufs=2) as rps_pool,
        tc.tile_pool(name="stg", bufs=2, space="PSUM") as stg_pool,
        tc.tile_pool(name="otp", bufs=1, space="PSUM") as ot_pool,
        tc.tile_pool(name="drp", bufs=1, space="PSUM") as dr_pool,
        tc.tile_pool(name="yp", bufs=1, space="PSUM") as y_pool,
    ):
        # ---- constants ----
        ones = consts.tile([128, 1], FP32)
        nc.vector.memset(ones[:], 1.0)
        # ind33[0] = 1 for cols 0:64, ind33[32] = 1 for cols 64:128 (rows
        # 1-31 zero). Rows {0,32} match the D matmul outputs' PSUM base
        # partitions, and both are legal 32-aligned SBUF write starts.
        ind33 = consts.tile([33, 128], FP32)
        nc.vector.memset(ind33[:], 0.0)
        nc.vector.memset(ind33[0:1, 0:64], 1.0)
        nc.vector.memset(ind33[32:33, 64:128], 1.0)
        # persistent reciprocal staging; rows 1-31 are read by the K=33
        # broadcast matmul against zero weights, so they must be finite.
        rec33 = consts.tile([33, QCHUNK], FP32)
        nc.vector.memset(rec33[:], 0.0)

        # ---- mask -> additive bias column [128, KB] ----
        mask_u8 = consts.tile([128, KB], mybir.dt.uint8)
        nc.sync.dma_start(mask_u8[:], m_d.rearrange("(kb p) -> p kb", p=128))
        mask_f = consts.tile([128, KB], FP32)
        nc.vector.tensor_copy(mask_f[:], mask_u8[:])
        bias = consts.tile([128, KB], FP32)
        nc.vector.tensor_scalar(
            bias[:], mask_f[:], 1.0e12, -1.0e12,
            op0=mybir.AluOpType.mult, op1=mybir.AluOpType.add,
        )

        # ---- big SBUF residents ----
        # qt/kt: chunk eb holds X^T[e in eb, s] at cols [eb*1024, +1024)
        qt = bigq.tile([128, KB * S], FP32)
        kt = bigq.tile([128, KB * S], FP32)
        wt = bigq.tile([128, KB * S], FP32)
        vn = bigq.tile([128, KB * E], FP32)

        qsrc = q_d.rearrange("(eb p) s -> eb p s", p=128)
        ksrc = k_d.rearrange("(eb p) s -> eb p s", p=128)
        wsrc = w_d.rearrange("(eb p) f -> eb p f", p=128)
        vsrc = v_d.rearrange("(kb p) e -> kb p e", p=128)
        for eb in range(KB):
            nc.sync.dma_start(qt[:, eb * S:(eb + 1) * S], qsrc[eb])
            nc.sync.dma_start(kt[:, eb * S:(eb + 1) * S], ksrc[eb])
        for kb in range(KB):
            nc.sync.dma_start(vn[:, kb * E:(kb + 1) * E], vsrc[kb])
        for eb in range(KB):
            nc.sync.dma_start(wt[:, eb * E:(eb + 1) * E], wsrc[eb])

        for qc in range(0, S, QCHUNK):
            ots_tiles = []
            for pr in range(H // 2):
                h0, h1 = 2 * pr, 2 * pr + 1
                ot = ot_pool.tile([128, QCHUNK], FP32, tag="ot")
                dr = dr_pool.tile([128, QCHUNK], FP32, tag="dr")
                for kb in range(KB):
                    stg = stg_pool.tile([128, 2 * QCHUNK], FP32, tag="stg")
                    # S_T = K_h Q_h^T, row-packed head pair
                    lhs0 = kt[0:64, pr * S + kb * 128: pr * S + (kb + 1) * 128]
                    rhs0 = qt[0:64, pr * S + qc: pr * S + qc + QCHUNK]
                    nc.tensor.matmul(stg[:, 0:QCHUNK], lhs0, rhs0,
                                     start=True, stop=True, tile_position=(0, 0))
                    lhs1 = kt[64:128, pr * S + kb * 128: pr * S + (kb + 1) * 128]
                    rhs1 = qt[64:128, pr * S + qc: pr * S + qc + QCHUNK]
                    nc.tensor.matmul(stg[:, QCHUNK:2 * QCHUNK], lhs1, rhs1,
                                     start=True, stop=True, tile_position=(64, 0))
                    # P = exp(S_T/8 + bias)
                    pt = p_pool.tile([128, 2 * QCHUNK], FP32, tag="pt")
                    nc.scalar.activation(
                        pt[:], stg[:], mybir.ActivationFunctionType.Exp,
                        bias=bias[:, kb:kb + 1], scale=0.125,
                    )
                    # O_T += V^T-style matmul, col-packed heads
                    st, sp = (kb == 0), (kb == KB - 1)
                    nc.tensor.matmul(
                        ot[0:64, :], vn[:, kb * E + h0 * D: kb * E + h0 * D + D],
                        pt[:, 0:QCHUNK], start=st, stop=sp, tile_position=(0, 0),
                        skip_group_check=True,
                    )
                    nc.tensor.matmul(
                        ot[64:128, :], vn[:, kb * E + h1 * D: kb * E + h1 * D + D],
                        pt[:, QCHUNK:2 * QCHUNK], start=st, stop=sp,
                        tile_position=(0, 64), skip_group_check=True,
                    )
                    # D += ones^T P
                    nc.tensor.matmul(
                        dr[0:1, :], ones[:], pt[:, 0:QCHUNK],
                        start=st, stop=sp, tile_position=(0, 0),
                        skip_group_check=True,
                    )
                    nc.tensor.matmul(
                        dr[32:33, :], ones[:], pt[:, QCHUNK:2 * QCHUNK],
                        start=st, stop=sp, tile_position=(0, 32),
                        skip_group_check=True,
                    )
                # normalize: OTs = ot * broadcast(1/D)
                nc.vector.reciprocal(rec33[0:1, :], dr[0:1, :])
                nc.vector.reciprocal(rec33[32:33, :], dr[32:33, :])
                rp = y_pool.tile([128, QCHUNK], FP32, tag="y")
                nc.tensor.matmul(rp[:], ind33[:], rec33[:], start=True, stop=True)
                rps = rps_pool.tile([128, QCHUNK], FP32, tag="rps")
                nc.vector.tensor_copy(rps[:], rp[:])
                ots = ots_pool.tile([128, QCHUNK], FP32, tag="ots")
                nc.vector.tensor_mul(ots[:], ot[:], rps[:])
                ots_tiles.append(ots)

            # projection for this q-half
            for j in range(QCHUNK // 128):
                qb = qc // 128 + j
                y = y_pool.tile([128, E], FP32, tag="y")
                for fc in range(0, E, QCHUNK):
                    for eb in range(KB):
                        nc.tensor.matmul(
                            y[:, fc:fc + QCHUNK],
                            ots_tiles[eb][:, j * 128:(j + 1) * 128],
                            wt[:, eb * E + fc: eb * E + fc + QCHUNK],
                            start=(eb == 0), stop=(eb == KB - 1),
                            skip_group_check=True,
                        )
                ysb = ysb_pool.tile([128, E], FP32, tag="ysb")
                nc.any.tensor_copy(ysb[:], y[:])
                nc.sync.dma_start(o_d[qb * 128:(qb + 1) * 128, :], ysb[:])


_CACHE = {}


def build():
    if "nc" not in _CACHE:
        nc = bacc.Bacc("TRN2", target_bir_lowering=False, debug=False,
                       enable_asserts=False)
        with tile.TileContext(nc) as tc:
            emit(tc)
        nc.compile()
        _CACHE["nc"] = nc
    return _CACHE["nc"]


def kernel(keys, values, queries, attention_mask, w_out, trace=False):
    nc = build()
    wT = np.ascontiguousarray(np.asarray(w_out, dtype=np.float32).T)
    in_maps = []
    for b in range(NCORES):
        in_maps.append({
            "qT": np.ascontiguousarray(np.asarray(queries[b], dtype=np.float32).T),
            "kT": np.ascontiguousarray(np.asarray(keys[b], dtype=np.float32).T),
            "v": np.ascontiguousarray(np.asarray(values[b], dtype=np.float32)),
            "m": np.ascontiguousarray(
                np.asarray(attention_mask[b]).reshape(S).astype(np.uint8)),
            "wT": wT,
        })
    res = run_bass_kernel_spmd(nc, in_maps, core_ids=list(range(NCORES)),
                               trace=trace)
    out = np.stack([res.results[b]["out"] for b in range(NCORES)], axis=0)
    if trace:
        _CACHE["last_results"] = res
    return out.astype(np.float32)
